# revision 1
# baseline (speedup 1.0000x reference)
"""CRF Viterbi decode kernel for Trainium2 (8 NeuronCores, SPMD data-parallel).

Problem: logits [256, 512, 128] f32, transitions [128, 128] f32,
sequence_lengths [256] i32 -> pred_ids [256, 512] i32.

Strategy (v4 — full-partition max-plus scan, ~2.38 ms/core in CoreSim vs
17.6 ms for the 32-partition baseline; DVE ~98% busy, its reduce stream
is the irreducible bound):
  - Shard batch 256 -> 32 per core (8 cores); device runs the UNFROZEN
    forward Viterbi max-plus scan and stores the pre-emission state
    trajectory M_t (M_0 = 0, S_t = M_t + x_t); the host adds logits back
    and runs the exact O(B*T*N) backpointer recompute + backtrack.
  - Per core the 32 batches split into 2 independent chains of BH=16 whose
    instructions interleave, hiding each chain's serial latency.
  - Chain layout: partitions p = jg*16 + b (jg in [0,8) groups of
    jl in [0,16) tags, j = jg*16 + jl), so all 128 partitions work:
      PE:   Sbc[p, i] = S_{t-1}[b, i] = (R^T @ M_{t-1}) + (R^T @ x_{t-1})
            via two accumulating matmuls with a 0/1 replication matrix R
      ACT:  copy Sbc PSUM -> SBUF (GPSIMD cannot read PSUM)
      Pool: sc[p, jl, i] = Sbc[p, i] + transG[p, jl, i]  (scores; 3 slices)
      DVE:  msc[p, jl] = max_i sc[p, jl, i]              (3 slices)
      PE:   8 selection matmuls regroup msc[(jg,b), jl] -> M_t[b, j] PSUM
      ACT:  copy M_t PSUM -> SBUF staging (also the traj store layout)
    DVE (all reduces) and Pool (all adds) are the balanced bottleneck pair;
    the jl-sliced adds/reduces let each reduce start as soon as its slice
    of the scores is written. (gpsimd tensor_tensor(max) would let Pool
    pre-fold the reduce, but the Pool engine has no TT-max opcode on HW —
    walrus rejects it — so DVE owns the whole reduce stream.)
  - Logits load and trajectory store are chunked (CH steps per DMA);
    startup DMAs overlap on two DGE queues with small leading slices, and
    the last chunk stores in pieces, so pipeline fill/drain costs ~10 us.
"""

import os
import sys

import numpy as np

sys.path.insert(0, "/opt/trn_rl_repo")

import concourse.bass as bass  # noqa: E402
import concourse.mybir as mybir  # noqa: E402
from concourse.tile import TileContext  # noqa: E402
from concourse.bass_utils import run_bass_kernel_spmd  # noqa: E402

B, T, N = 256, 512, 128
NCORES = 8
BC = B // NCORES  # 32 batches per core
BH = 16  # batches per chain (2 chains)
NG = 8  # j-groups per chain
JL = N // NG  # 16 tags per group
CH = 32  # time-chunk (DMA batching)

_PROGRAM = None
_TSTEPS = int(os.environ.get("CRF_TSTEPS", str(T)))


# The last _FOLD jl's get _NFOLD levels of gpsimd max-pre-fold
# (i 128 -> 128>>_NFOLD) before the DVE reduce, shifting reduce work from
# DVE (the bottleneck) to Pool's slack.
_FOLD = int(os.environ.get("CRF_FOLD", "0"))
_NFOLD = int(os.environ.get("CRF_NFOLD", "1"))


def _splits(env, default):
    v = os.environ.get(env, default)
    ws = [int(x) for x in v.split(",")]
    assert sum(ws) == JL - _FOLD, (env, v, _FOLD)
    return ws


_ADD_SPLITS = _splits("CRF_ADDS", "3,6,7")
_RED_SPLITS = _splits("CRF_REDS", "3,6,7")


def _build_program(tsteps=T):
    nc = bass.Bass("TRN2")
    f32 = mybir.dt.float32

    logits = nc.dram_tensor("logits", [BC, T, N], f32, kind="ExternalInput")
    # transG[p=(jg,b), (jl, i)] = trans[i, jg*JL+jl] (independent of b)
    transG = nc.dram_tensor("transG", [128, JL * N], f32, kind="ExternalInput")
    # repmat[b, jg*BH+b'] = 1{b'==b}
    repmat = nc.dram_tensor("repmat", [BH, 128], f32, kind="ExternalInput")
    # ident for selection matmuls
    ident = nc.dram_tensor("ident", [128, 128], f32, kind="ExternalInput")
    # traj[t, b, i] = M_t[b, i] (pre-emission state; M_0 = 0)
    traj = nc.dram_tensor("traj", [T, BC, N], f32, kind="ExternalOutput")

    nch = (tsteps + CH - 1) // CH  # chunks

    with TileContext(nc) as tc:
        with (
            tc.tile_pool(name="persist", bufs=1) as pp,
            tc.tile_pool(name="xc", bufs=2) as xp,
            tc.tile_pool(name="stg", bufs=2) as sp,
            tc.tile_pool(name="sc", bufs=2) as scp,
            tc.tile_pool(name="msc", bufs=2) as mp,
            tc.tile_pool(name="psA", bufs=2, space=bass.MemorySpace.PSUM) as psA,
            tc.tile_pool(name="psB", bufs=2, space=bass.MemorySpace.PSUM) as psB,
        ):
            # startup DMAs spread across DGE queues so they overlap; the
            # first add-slice's transG rows lead so the pipeline can start
            tG = pp.tile([128, JL * N], f32)
            w0 = _ADD_SPLITS[0]
            nc.scalar.dma_start(out=tG[:, : w0 * N], in_=transG[:, : w0 * N])
            tR = pp.tile([BH, 128], f32)
            nc.scalar.dma_start(out=tR[:], in_=repmat[:, :])
            tI = pp.tile([128, 128], f32)
            nc.scalar.dma_start(out=tI[:], in_=ident[:, :])
            nc.scalar.dma_start(out=tG[:, w0 * N :], in_=transG[:, w0 * N :])
            tG3 = tG[:].rearrange("p (jl i) -> p jl i", i=N)

            # per-chain stg/xc tiles [BH, CH*N] at base partition 0;
            # logits chunks are prefetched one chunk ahead of use
            def load_chunk(c):
                t0 = c * CH
                steps = min(CH, tsteps - t0)
                tiles = []
                for ch in range(2):
                    xt = xp.tile(
                        [BH, CH * N], f32, tag=f"xc{ch}", name=f"xc{ch}"
                    )
                    # chunk 0 gates the pipeline start: land the first
                    # few steps in a small leading DMA
                    cuts = [0, 2, steps] if c == 0 and steps > 2 else [0, steps]
                    for s0, s1 in zip(cuts, cuts[1:]):
                        nc.sync.dma_start(
                            out=xt[:, s0 * N : s1 * N],
                            in_=logits[
                                ch * BH : (ch + 1) * BH, t0 + s0 : t0 + s1, :
                            ].rearrange("b t i -> b (t i)"),
                        )
                    tiles.append(xt)
                return tiles

            prev_stg = None  # previous chunk's staging tiles (per chain)
            prev_xc = None
            next_xc = load_chunk(0)
            for c in range(nch):
                t0 = c * CH
                steps = min(CH, tsteps - t0)
                xc = next_xc
                next_xc = load_chunk(c + 1) if c + 1 < nch else None
                stg = [
                    sp.tile([BH, CH * N], f32, tag=f"stg{ch}", name=f"stg{ch}")
                    for ch in range(2)
                ]
                if c == 0:
                    for ch in range(2):
                        # M_0 = 0
                        nc.vector.memset(stg[ch][:, 0:N], 0.0)
                for s in range(steps):
                    t = t0 + s
                    if t == 0:
                        continue
                    # slices for M_{t-1} and x_{t-1}
                    if s == 0:
                        pstg, pxc, ps_ = prev_stg, prev_xc, CH - 1
                    else:
                        pstg, pxc, ps_ = stg, xc, s - 1
                    for ch in range(2):
                        mprev = pstg[ch][:, ps_ * N : (ps_ + 1) * N]
                        xprev = pxc[ch][:, ps_ * N : (ps_ + 1) * N]

                        msc = mp.tile([128, JL], f32, tag=f"msc{ch}")
                        sbc = psA.tile([128, N], f32, tag=f"sbc{ch}")
                        nc.tensor.matmul(
                            sbc[:], tR[:], mprev, start=True, stop=False
                        )
                        nc.tensor.matmul(
                            sbc[:], tR[:], xprev, start=False, stop=True
                        )
                        # GPSIMD can't read PSUM: stage Sbc into SBUF via ACT
                        sbs = mp.tile([128, N], f32, tag=f"sbs{ch}", name="sbs")
                        nc.scalar.activation(
                            out=sbs[:],
                            in_=sbc[:],
                            func=mybir.ActivationFunctionType.Copy,
                        )
                        sc = scp.tile([128, JL * N], f32, tag=f"sc{ch}")
                        sc3 = sc[:].rearrange("p (jl i) -> p jl i", i=N)

                        def sbs_bcast(njl):
                            return bass.AP(
                                sbs[:].tensor,
                                sbs[:].offset,
                                [list(sbs[:].ap[0]), [0, njl], list(sbs[:].ap[1])],
                            )

                        ufj = JL - _FOLD
                        lo = 0
                        for w in _ADD_SPLITS:
                            hi = lo + w
                            nc.gpsimd.tensor_tensor(
                                out=sc3[:, lo:hi, :],
                                in0=sbs_bcast(w),
                                in1=tG3[:, lo:hi, :],
                                op=mybir.AluOpType.add,
                            )
                            lo = hi
                        if _FOLD:
                            nc.gpsimd.tensor_tensor(
                                out=sc3[:, ufj:JL, :],
                                in0=sbs_bcast(_FOLD),
                                in1=tG3[:, ufj:JL, :],
                                op=mybir.AluOpType.add,
                            )
                            # fold i 128 -> 128>>_NFOLD on gpsimd
                            fw = N
                            src3 = sc3[:, ufj:JL, :]
                            for lvl in range(_NFOLD):
                                fw //= 2
                                dst = mp.tile(
                                    [128, _FOLD * fw], f32,
                                    tag=f"scf{ch}_{lvl}", name="scF",
                                )
                                dst3 = dst[:].rearrange(
                                    "p (jl i) -> p jl i", i=fw
                                )
                                nc.gpsimd.tensor_tensor(
                                    out=dst3,
                                    in0=src3[:, :, 0:fw],
                                    in1=src3[:, :, fw : 2 * fw],
                                    op=mybir.AluOpType.max,
                                )
                                src3 = dst3
                            nc.vector.tensor_reduce(
                                out=msc[:, ufj:JL],
                                in_=src3,
                                axis=mybir.AxisListType.X,
                                op=mybir.AluOpType.max,
                            )
                        lo = 0
                        for w in _RED_SPLITS:
                            hi = lo + w
                            nc.vector.tensor_reduce(
                                out=msc[:, lo:hi],
                                in_=sc3[:, lo:hi, :],
                                axis=mybir.AxisListType.X,
                                op=mybir.AluOpType.max,
                            )
                            lo = hi
                        mbm = psB.tile([BH, N], f32, tag=f"mbm{ch}")
                        for jg in range(NG):
                            nc.tensor.matmul(
                                mbm[:, jg * JL : (jg + 1) * JL],
                                tI[:, jg * BH : jg * BH + BH],
                                msc[:],
                                start=True,
                                stop=True,
                            )
                        nc.scalar.activation(
                            out=stg[ch][:, s * N : (s + 1) * N],
                            in_=mbm[:],
                            func=mybir.ActivationFunctionType.Copy,
                        )
                # traj[t0:t0+steps] <- stg ; dst loops reordered to (b, t, i).
                # Last chunk: store in halves so the first half drains while
                # the final steps still compute.
                cuts = (
                    [0, steps // 2, 3 * steps // 4, steps]
                    if c == nch - 1 and steps > 3
                    else [0, steps]
                )
                for ch in range(2):
                    for s0, s1 in zip(cuts, cuts[1:]):
                        nc.sync.dma_start(
                            out=bass.AP(
                                traj.ap().tensor,
                                (t0 + s0) * BC * N + ch * BH * N,
                                [[N, BH], [BC * N, s1 - s0], [1, N]],
                            ),
                            in_=stg[ch][:, s0 * N : s1 * N],
                        )
                prev_stg, prev_xc = stg, xc

    return nc


def _get_program():
    global _PROGRAM
    if _PROGRAM is None:
        nc = _build_program(_TSTEPS)
        # Split multi-wait instructions (TRN2 allows 1 sync wait per
        # instruction); the axon exec path ships raw BIR and skips this
        # bacc finalization, so run it explicitly.
        from concourse.bass_utils import bass_rust

        bass_rust.generate_event_semaphores(nc)
        _PROGRAM = nc
    return _PROGRAM


def _aux_inputs(transitions):
    transT = np.ascontiguousarray(transitions.T)  # [j, i]
    transG = np.empty((128, JL * N), dtype=np.float32)
    for jg in range(NG):
        row = transT[jg * JL : (jg + 1) * JL, :].reshape(-1)
        transG[jg * BH : (jg + 1) * BH, :] = row[None, :]
    repmat = np.tile(np.eye(BH, dtype=np.float32), (1, NG))  # [16, 128]
    ident = np.eye(128, dtype=np.float32)
    return {"transG": transG, "repmat": repmat, "ident": ident}


_OUT_NAMES = ["traj"]


def _make_in_map(logits, transitions, core):
    aux = _aux_inputs(transitions)
    sl = slice(core * BC, (core + 1) * BC)
    return {"logits": np.ascontiguousarray(logits[sl]), **aux}


def _traj_from_out(outs, logits_shard):
    # stored traj is M_t (M_0 = 0); S_t = M_t + x_t
    return outs["traj"] + np.moveaxis(logits_shard, 0, 1)


def _forward_device(logits, transitions):
    nc = _get_program()
    in_maps = [_make_in_map(logits, transitions, c) for c in range(NCORES)]
    res = run_bass_kernel_spmd(nc, in_maps, core_ids=list(range(NCORES)))
    trajM = np.concatenate([r["traj"] for r in res.results], axis=1)  # [T, B, N]
    traj = trajM + np.moveaxis(logits, 0, 1)
    return traj, res


def _forward_numpy(logits, transitions):
    state = logits[:, 0, :].copy()
    traj = np.empty((T, B, N), dtype=np.float32)
    traj[0] = state
    transT = transitions.T[None]  # [1, j, i]
    for t in range(1, T):
        state = (state[:, None, :] + transT).max(-1) + logits[:, t, :]
        traj[t] = state
    return traj


def kernel(logits, transitions, sequence_lengths, _results_hook=None):
    logits = np.asarray(logits, dtype=np.float32)
    transitions = np.asarray(transitions, dtype=np.float32)
    sequence_lengths = np.asarray(sequence_lengths, dtype=np.int32)

    res = None
    try:
        traj, res = _forward_device(logits, transitions)
    except Exception as exc:  # device/compile failure: exact numpy fallback
        sys.stderr.write(f"device path failed ({exc!r}); numpy fallback\n")
        traj = _forward_numpy(logits, transitions)
    if _results_hook is not None:
        _results_hook(res)

    # ---- host backward pass (exact; O(B*T*N)) ----
    # Device trajectory is UNFROZEN; the reference's frozen state at step t
    # equals traj[min(t, L-1)]. All backward reads below use indices < L-1,
    # except last_tag which reads the clamped final state.
    L = sequence_lengths.astype(np.int64)
    cur = traj[L - 1, np.arange(B)].argmax(axis=1)  # last_tag [B]
    tags = np.empty((B, T), dtype=np.int64)
    tags[:, T - 1] = cur
    for i in range(T - 2, -1, -1):
        # step i used state_i (pre-update); active iff (i+1) < L
        cand = traj[i] + transitions[:, cur].T  # [B, N]
        new = cand.argmax(axis=1)
        cur = np.where((i + 1) < L, new, cur)
        tags[:, i] = cur
    mask = np.arange(T)[None, :] < L[:, None]
    return (tags * mask).astype(np.int32)

